# revision 2
# baseline (speedup 1.0000x reference)
"""Trainium2 Bass kernel for nn_BasicBlock_1w8a_q (quantized ResNet BasicBlock,
1-bit weights / 8-bit activations).

Strategy:
 - Pure data parallel over 8 NeuronCores: batch 32 -> 4 images per core.
 - Layout: channels C=128 on SBUF partitions, spatial on the free dim.
 - Each 3x3 conv with sign(+-1) weights = 9 shifted matmuls accumulating in
   PSUM (lhsT = [C_in, C_out] weight slice, rhs = shifted padded input view).
 - /4 is folded into the weights (+-0.25, exact in fp16; power-of-2 scaling
   commutes with IEEE rounding so psum == conv/4 bit-for-bit).
 - conv1 runs in 2 fp16 passes (x = hi + lo split, ~f32-exact);
   conv2 runs in 1 fp16 pass (x1 is integers in [-7,7]: exact).
 - All rounds use the DVE f32->int32 cast which is exact round-half-to-even
   (verified on HW) == jnp.round semantics; floor(z) = RNE(z - 0.5) for
   non-integer z.
 - BN folding / per-channel constants are computed on host mirroring the
   reference's f32 op order exactly.
"""

import os

import numpy as np

import concourse.bacc as bacc
import concourse.tile as tile
import concourse.mybir as mybir
from concourse.bass_utils import run_bass_kernel_spmd
from concourse.mybir import AluOpType as Op

F32 = mybir.dt.float32
F16 = mybir.dt.float16
I32 = mybir.dt.int32
I8 = mybir.dt.int8

B, C, H, W = 32, 128, 56, 56
NCORES = 8
BS = B // NCORES            # images per core
HP, WP = H + 2, W + 2       # padded spatial
HB = 8                      # output rows per chunk
NCH = H // HB               # chunks per image
CHUNK = HB * W              # 448 columns per psum chunk
SHIFTS = [(ky, kx) for ky in range(3) for kx in range(3)]

f32 = np.float32


# ---------------------------------------------------------------------------
# Host-side prep: mirrors the reference's f32 op order exactly.
# ---------------------------------------------------------------------------

def _qfn(x, prec):
    # x + (round(x*n)/n - x), all in f32 like the reference
    n = f32(2.0 ** prec - 1.0)
    q = (np.round(x * n) / n).astype(f32)
    return (x + (q - x)).astype(f32)


def _my_quantize(x, prec):
    T = np.clip(np.max(np.abs(x)), f32(1e-10), f32(255.0)).astype(f32)
    return (_qfn((np.clip(x, -T, T) / T).astype(f32), prec) * T).astype(f32)


def _bn_consts(gamma, beta, mean, var):
    """Returns (bw [C] int-valued f32, bb [C] int-valued f32, T_w scalar f32)."""
    gamma, beta, mean, var = (a.astype(f32) for a in (gamma, beta, mean, var))
    std = np.sqrt(var + f32(1e-5)).astype(f32)
    w = (gamma / std).astype(f32)
    bq = (beta - w * mean).astype(f32)
    T_w = np.max(np.abs(w)).astype(f32)
    bw = (_qfn((np.clip(w, -T_w, T_w) / T_w).astype(f32), 3) * f32(7.0)).astype(f32)
    qb = _my_quantize(bq, 14)
    # bb = round(qb * 7.0 * 1023.0 / 4032.0 * 7.0 / T_w), left-assoc f32
    t = (qb * f32(7.0)).astype(f32)
    t = (t * f32(1023.0)).astype(f32)
    t = (t / f32(4032.0)).astype(f32)
    t = (t * f32(7.0)).astype(f32)
    t = (t / T_w).astype(f32)
    bb = np.round(t).astype(f32)
    return bw, bb, T_w


def _sc_th(T_w):
    # sc = round(1023.0/4032.0 * 7.0 / T_w); Th = round(7.0*1023.0/4032.0*7.0/T_w)
    a = (f32(1023.0) / f32(4032.0)).astype(f32)
    a = (a * f32(7.0)).astype(f32)
    sc = np.round((a / T_w).astype(f32)).astype(f32)
    b2 = (f32(7.0) * f32(1023.0)).astype(f32)
    b2 = (b2 / f32(4032.0)).astype(f32)
    b2 = (b2 * f32(7.0)).astype(f32)
    Th = np.round((b2 / T_w).astype(f32)).astype(f32)
    return sc, Th


def _ref_final_map(k, Th):
    # reference: round(clip(k,-Th,Th)/Th*7.0) in f32
    kk = np.clip(f32(k), -Th, Th).astype(f32)
    return float(np.round(((kk / Th).astype(f32) * f32(7.0)).astype(f32)))


def _pick_scale(Th):
    """Find s (f32) such that clip(RNE(k*s), -7, 7) == round(clip(k)/Th*7)
    for every integer k. RNE == np.round on device (verified)."""
    Thf = f32(Th)
    base = f32(f32(7.0) / Thf)
    cands = [base,
             np.nextafter(base, f32(np.inf), dtype=f32),
             np.nextafter(base, f32(-np.inf), dtype=f32),
             f32(7.0 / float(Th))]
    kmax = int(Th) + 3
    for s in cands:
        ok = True
        for k in range(-kmax, kmax + 1):
            got = float(np.clip(np.round(f32(f32(k) * s)), -7.0, 7.0))
            want = _ref_final_map(k, Thf)
            if got != want:
                ok = False
                break
        if ok:
            return f32(s)
    raise AssertionError(f"no matching scale for Th={Th}")


def _host_prep(x, w1, w2, g1, b1, m1, v1, g2, b2, m2, v2):
    w1 = w1.astype(f32)
    w2 = w2.astype(f32)
    sw1 = np.abs(w1).mean(axis=(1, 2, 3), dtype=np.float32).astype(f32)
    sw2 = np.abs(w2).mean(axis=(1, 2, 3), dtype=np.float32).astype(f32)
    bw1, bb1, Tw1 = _bn_consts(g1, b1, m1, v1)
    bw2, bb2, Tw2 = _bn_consts(g2, b2, m2, v2)
    sc1, Th1 = _sc_th(Tw1)
    sc2, Th2 = _sc_th(Tw2)
    s1 = _pick_scale(Th1)
    s2 = _pick_scale(Th2)

    # weight tiles: lhsT[s][ci, co] = sign(w)[co, ci, ky, kx] * 0.25, fp16
    def wtiles(w):
        sg = (np.sign(w) * 0.25).astype(np.float16)  # [O, I, 3, 3]
        t = np.empty((C, 9, C), np.float16)          # [ci, s, co]
        for s, (ky, kx) in enumerate(SHIFTS):
            t[:, s, :] = sg[:, :, ky, kx].T
        return t

    cv = np.zeros((C, 12), f32)
    cv[:, 0] = sw1
    cv[:, 1] = bw1
    cv[:, 2] = bb1
    cv[:, 3] = sc1
    cv[:, 4] = s1
    cv[:, 5] = sw2
    cv[:, 6] = bw2
    cv[:, 7] = bb2
    cv[:, 8] = sc2
    cv[:, 9] = s2
    return wtiles(w1), wtiles(w2), cv


# ---------------------------------------------------------------------------
# Device program
# ---------------------------------------------------------------------------

_prog_cache = {}


def _build_program():
    if "nc" in _prog_cache:
        return _prog_cache["nc"]
    nc = bacc.Bacc("TRN2", target_bir_lowering=False, debug=False,
                   num_devices=NCORES)
    d_x = nc.dram_tensor("xt", [C, BS, H, W], F32, kind="ExternalInput").ap()
    d_w1 = nc.dram_tensor("w1s", [C, 9, C], F16, kind="ExternalInput").ap()
    d_w2 = nc.dram_tensor("w2s", [C, 9, C], F16, kind="ExternalInput").ap()
    d_cv = nc.dram_tensor("cv", [C, 12], F32, kind="ExternalInput").ap()
    d_o = nc.dram_tensor("ot", [C, BS, H, W], I8, kind="ExternalOutput").ap()

    with tile.TileContext(nc) as tc:
        with tc.tile_pool(name="const", bufs=1) as const, \
             tc.tile_pool(name="pads", bufs=1) as pads, \
             tc.tile_pool(name="xin", bufs=2) as xin, \
             tc.tile_pool(name="tmp", bufs=3) as tmp, \
             tc.tile_pool(name="outp", bufs=1) as outp, \
             tc.tile_pool(name="psum", bufs=4, space="PSUM") as psum:

            cv = const.tile([C, 12], F32)
            nc.sync.dma_start(out=cv, in_=d_cv)
            w1 = const.tile([C, 9, C], F16)
            nc.sync.dma_start(out=w1, in_=d_w1)
            w2 = const.tile([C, 9, C], F16)
            nc.sync.dma_start(out=w2, in_=d_w2)

            A1, B1, BB1, SC1, S1 = (cv[:, i:i + 1] for i in range(5))
            A2, B2, BB2, SC2, S2 = (cv[:, i:i + 1] for i in range(5, 10))

            xh = pads.tile([C, BS, HP, WP], F16)
            xl = pads.tile([C, BS, HP, WP], F16)
            x1p = pads.tile([C, BS, HP, WP], F16)
            # zero the one-pixel borders (padding) on gpsimd
            for buf in (xh, xl, x1p):
                for b in range(BS):
                    nc.gpsimd.memset(buf[:, b, 0, :], 0.0)
                    nc.gpsimd.memset(buf[:, b, HP - 1, :], 0.0)
                    nc.gpsimd.memset(buf[:, b, 1:HP - 1, 0], 0.0)
                    nc.gpsimd.memset(buf[:, b, 1:HP - 1, WP - 1], 0.0)

            out_sb = outp.tile([C, BS, H, W], I8)

            # ---- load + hi/lo split (per image) ----
            xfs = []
            for b in range(BS):
                xf = xin.tile([C, H, W], F32, tag="xf")
                nc.sync.dma_start(out=xf, in_=d_x[:, b])
                nc.vector.tensor_copy(out=xh[:, b, 1:1 + H, 1:1 + W], in_=xf[:])
                nc.vector.scalar_tensor_tensor(
                    out=xl[:, b, 1:1 + H, 1:1 + W], in0=xf[:], scalar=1.0,
                    in1=xh[:, b, 1:1 + H, 1:1 + W],
                    op0=Op.mult, op1=Op.subtract)
                xfs.append(xf)

            # ---- stage 1: conv1 (hi+lo) + bn1 + residual + requant -> x1p ----
            for b in range(BS):
                for j in range(NCH):
                    r0 = j * HB
                    ps = psum.tile([C, CHUNK], F32, tag="ps")
                    for s, (ky, kx) in enumerate(SHIFTS):
                        rh = xh[:, b, r0 + ky:r0 + ky + HB, kx:kx + W]
                        rl = xl[:, b, r0 + ky:r0 + ky + HB, kx:kx + W]
                        nc.tensor.matmul(ps[:], w1[:, s, :], rh,
                                         start=(s == 0), stop=False)
                        nc.tensor.matmul(ps[:], w1[:, s, :], rl,
                                         start=False, stop=(s == 8))
                    # y = RNE(conv/4)  (psum already == conv/4)
                    y = tmp.tile([C, CHUNK], I32, tag="y")
                    nc.vector.tensor_scalar(out=y, in0=ps[:], scalar1=1.0,
                                            scalar2=None, op0=Op.mult)
                    # t = floor(y*sw1) = RNE(y*sw1 - 0.5)
                    t = tmp.tile([C, CHUNK], I32, tag="t")
                    nc.vector.tensor_scalar(out=t, in0=y[:], scalar1=A1,
                                            scalar2=0.5, op0=Op.mult,
                                            op1=Op.subtract)
                    # r = RNE(x*sc1 + bb1)
                    r = tmp.tile([C, CHUNK], I32, tag="r")
                    nc.vector.tensor_scalar(
                        out=r, in0=xfs[b][:, r0:r0 + HB, :], scalar1=SC1,
                        scalar2=BB1, op0=Op.mult, op1=Op.add)
                    # v = t*bw1 + r
                    v = tmp.tile([C, CHUNK], F32, tag="v")
                    nc.vector.scalar_tensor_tensor(out=v, in0=t[:], scalar=B1,
                                                   in1=r[:], op0=Op.mult,
                                                   op1=Op.add)
                    # u = RNE(v*s1);  x1 = clip(u, -7, 7) -> fp16 into x1p
                    u = tmp.tile([C, CHUNK], I32, tag="u")
                    nc.vector.tensor_scalar(out=u, in0=v[:], scalar1=S1,
                                            scalar2=None, op0=Op.mult)
                    nc.vector.tensor_scalar(
                        out=x1p[:, b, 1 + r0:1 + r0 + HB, 1:1 + W], in0=u[:],
                        scalar1=7.0, scalar2=-7.0, op0=Op.min, op1=Op.max)

            # ---- stage 2: conv2 + bn2 + residual(x1) + requant -> out ----
            for b in range(BS):
                for j in range(NCH):
                    r0 = j * HB
                    ps = psum.tile([C, CHUNK], F32, tag="ps")
                    for s, (ky, kx) in enumerate(SHIFTS):
                        rr = x1p[:, b, r0 + ky:r0 + ky + HB, kx:kx + W]
                        nc.tensor.matmul(ps[:], w2[:, s, :], rr,
                                         start=(s == 0), stop=(s == 8))
                    # y2 = RNE(conv2/4) -- exact (ties land on RNE)
                    y2 = tmp.tile([C, CHUNK], I32, tag="y2")
                    nc.vector.tensor_scalar(out=y2, in0=ps[:], scalar1=1.0,
                                            scalar2=None, op0=Op.mult)
                    # t2 = floor(y2*sw2)
                    t2 = tmp.tile([C, CHUNK], I32, tag="t2")
                    nc.vector.tensor_scalar(out=t2, in0=y2[:], scalar1=A2,
                                            scalar2=0.5, op0=Op.mult,
                                            op1=Op.subtract)
                    # r2 = x1*sc2 + bb2 (exact integers; ACT engine, off DVE)
                    r2 = tmp.tile([C, CHUNK], F32, tag="r2")
                    nc.scalar.activation(
                        out=r2, in_=x1p[:, b, 1 + r0:1 + r0 + HB, 1:1 + W],
                        func=mybir.ActivationFunctionType.Identity,
                        bias=BB2, scale=SC2)
                    # v2 = t2*bw2 + r2
                    v2 = tmp.tile([C, CHUNK], F32, tag="v2")
                    nc.vector.scalar_tensor_tensor(out=v2, in0=t2[:],
                                                   scalar=B2, in1=r2[:],
                                                   op0=Op.mult, op1=Op.add)
                    # u2 = RNE(v2*s2); out = clip(u2, -7, 7) -> int8
                    u2 = tmp.tile([C, CHUNK], I32, tag="u2")
                    nc.vector.tensor_scalar(out=u2, in0=v2[:], scalar1=S2,
                                            scalar2=None, op0=Op.mult)
                    nc.vector.tensor_scalar(
                        out=out_sb[:, b, r0:r0 + HB, :], in0=u2[:],
                        scalar1=7.0, scalar2=-7.0, op0=Op.min, op1=Op.max)

            nc.sync.dma_start(out=d_o, in_=out_sb[:])

    nc.compile()
    _prog_cache["nc"] = nc
    return nc


# ---------------------------------------------------------------------------
# Entry point
# ---------------------------------------------------------------------------

last_results = None


def kernel(x, w1, w2, gamma1, beta1, mean1, var1,
           gamma2, beta2, mean2, var2):
    global last_results
    w1t, w2t, cv = _host_prep(x, w1, w2, gamma1, beta1, mean1, var1,
                              gamma2, beta2, mean2, var2)
    nc = _build_program()

    in_maps = []
    for i in range(NCORES):
        shard = np.ascontiguousarray(
            x[i * BS:(i + 1) * BS].astype(f32).transpose(1, 0, 2, 3))
        in_maps.append({"xt": shard, "w1s": w1t, "w2s": w2t, "cv": cv})

    trace = bool(int(os.environ.get("KERNEL_TRACE", "0")))
    kwargs = {}
    if trace:
        # no artifact bucket in this container; neutralize the upload step
        import concourse.bass_utils as _bu
        _bu.upload_artifacts = lambda tmpdir: ""
        kwargs["tmpdir"] = os.environ.get("KERNEL_TRACE_DIR", "/tmp/ktrace")
        os.makedirs(kwargs["tmpdir"], exist_ok=True)
    res = run_bass_kernel_spmd(nc, in_maps, core_ids=list(range(NCORES)),
                               trace=trace, **kwargs)
    last_results = res

    out = np.empty((B, C, H, W), np.float32)
    for i in range(NCORES):
        out[i * BS:(i + 1) * BS] = \
            res.results[i]["ot"].astype(np.float32).transpose(1, 0, 2, 3)
    return out


# revision 5
# speedup vs baseline: 1.0092x; 1.0092x over previous
"""Trainium2 Bass kernel for nn_BasicBlock_1w8a_q (quantized ResNet BasicBlock,
1-bit weights / 8-bit activations).

Strategy:
 - Pure data parallel over 8 NeuronCores: batch 32 -> 4 images per core.
 - Layout: channels C=128 on SBUF partitions, spatial on the free dim.
 - Each 3x3 conv with sign(+-1) weights = 9 shifted matmuls accumulating in
   PSUM (lhsT = [C_in, C_out] weight slice, rhs = shifted padded input view).
 - /4 is folded into the weights (+-0.25, exact in fp16; power-of-2 scaling
   commutes with IEEE rounding so psum == conv/4 bit-for-bit).
 - conv1 runs in 2 fp16 passes (x = hi + lo split, ~f32-exact);
   conv2 runs in 1 fp16 pass (x1 is integers in [-7,7]: exact).
 - Rounds use the DVE f32->int32 cast (exact round-half-to-even, verified on
   HW, == jnp.round) or the +1.5*2^23 magic-constant trick (fp32 add rounds
   to integer by RNE), which lets the ScalarE (ACT) do rounds too.
 - BN folding / per-channel constants are computed on host mirroring the
   reference's f32 op order; data-dependent fused scales are grid-verified
   on host against the reference mapping before use.
"""

import os

import numpy as np

import concourse.bacc as bacc
import concourse.tile as tile
import concourse.mybir as mybir
from concourse.bass_utils import run_bass_kernel_spmd
from concourse.mybir import AluOpType as Op

F32 = mybir.dt.float32
F16 = mybir.dt.float16
I32 = mybir.dt.int32
I8 = mybir.dt.int8
IDENT = mybir.ActivationFunctionType.Identity

B, C, H, W = 32, 128, 56, 56
NCORES = 8
BS = B // NCORES            # images per core
HP, WP = H + 2, W + 2       # padded spatial
HB = 8                      # output rows per psum chunk
NCH = H // HB               # chunks per image (7)
CHUNK = HB * W              # 448 columns per psum chunk
BANK = 512                  # fp32 slots per PSUM bank
GROUPS = [(0, 4), (4, 3)]   # (first chunk, n chunks) per psum group
SHIFTS = [(ky, kx) for ky in range(3) for kx in range(3)]
MAGIC = float(np.float32(12582912.0))   # 1.5 * 2^23, even integer

f32 = np.float32


# ---------------------------------------------------------------------------
# Host-side prep: mirrors the reference's f32 op order exactly.
# ---------------------------------------------------------------------------

def _qfn(x, prec):
    n = f32(2.0 ** prec - 1.0)
    q = (np.round(x * n) / n).astype(f32)
    return (x + (q - x)).astype(f32)


def _my_quantize(x, prec):
    T = np.clip(np.max(np.abs(x)), f32(1e-10), f32(255.0)).astype(f32)
    return (_qfn((np.clip(x, -T, T) / T).astype(f32), prec) * T).astype(f32)


def _bn_consts(gamma, beta, mean, var):
    gamma, beta, mean, var = (a.astype(f32) for a in (gamma, beta, mean, var))
    std = np.sqrt(var + f32(1e-5)).astype(f32)
    w = (gamma / std).astype(f32)
    bq = (beta - w * mean).astype(f32)
    T_w = np.max(np.abs(w)).astype(f32)
    bw = (_qfn((np.clip(w, -T_w, T_w) / T_w).astype(f32), 3) * f32(7.0)).astype(f32)
    qb = _my_quantize(bq, 14)
    t = (qb * f32(7.0)).astype(f32)
    t = (t * f32(1023.0)).astype(f32)
    t = (t / f32(4032.0)).astype(f32)
    t = (t * f32(7.0)).astype(f32)
    t = (t / T_w).astype(f32)
    bb = np.round(t).astype(f32)
    return bw, bb, T_w


def _sc_th(T_w):
    a = (f32(1023.0) / f32(4032.0)).astype(f32)
    a = (a * f32(7.0)).astype(f32)
    sc = np.round((a / T_w).astype(f32)).astype(f32)
    b2 = (f32(7.0) * f32(1023.0)).astype(f32)
    b2 = (b2 / f32(4032.0)).astype(f32)
    b2 = (b2 * f32(7.0)).astype(f32)
    Th = np.round((b2 / T_w).astype(f32)).astype(f32)
    return sc, Th


def _ref_final_vec(k, Th):
    # reference: round(clip(k,-Th,Th)/Th*7.0) elementwise in f32
    kk = np.clip(k.astype(f32), -Th, Th).astype(f32)
    return np.round(((kk / Th).astype(f32) * f32(7.0)).astype(f32))


def _scale_cands(Th):
    base = f32(f32(7.0) / f32(Th))
    out = [base]
    up, dn = base, base
    for _ in range(8):
        up = np.nextafter(up, f32(np.inf), dtype=f32)
        dn = np.nextafter(dn, f32(-np.inf), dtype=f32)
        out += [up, dn]
    return out


def _pick_scale(Th):
    """s (f32) with clip(RNE(k*s),-7,7) == round(clip(k,-Th,Th)/Th*7) for all
    integer k (device RNE == np.round, verified on HW)."""
    kk = np.arange(-3000, 3001, dtype=f32)
    want = _ref_final_vec(kk, f32(Th))
    for s in _scale_cands(Th):
        got = np.clip(np.round((kk * s).astype(f32)), -7.0, 7.0)
        if np.array_equal(got, want):
            return f32(s)
    raise AssertionError(f"no matching scale for Th={Th}")


def _pick_fused_stage2(bw2, bb2, sc2, Th2):
    """Stage-2 fusion: u2 = RNE(t2*(bw2*s) + (x1*(sc2*s) + bb2*s)) must equal
    ref round(clip(v2)/Th2*7) (then clip +-7) for v2 = t2*bw2 + x1*sc2 + bb2.
    Returns (B2s, rscale, rbias, s) all f32, host-verified over a full grid
    with a tie-margin so ACT fma-vs-two-round ambiguity cannot flip a round.
    """
    t2g = np.arange(-640, 641, dtype=f32)[None, :, None]       # [1,T,1]
    x1g = np.arange(-7, 8, dtype=f32)[None, None, :]           # [1,1,15]
    bwc = bw2.astype(f32)[:, None, None]                       # [C,1,1]
    bbc = bb2.astype(f32)[:, None, None]
    v2 = (t2g * bwc + x1g * f32(sc2) + bbc).astype(f32)        # exact ints
    want = np.clip(_ref_final_vec(v2, f32(Th2)), -7.0, 7.0)
    base = f32(f32(7.0) / f32(Th2))
    for j in range(0, 60):
        s = f32(base * f32(1.0 + j * 2.0 ** -19))
        B2s = (bw2 * s).astype(f32)
        rscale = f32(f32(sc2) * s)
        rbias = (bb2 * s).astype(f32)
        # device sim (two-round form)
        r2s = ((x1g * rscale).astype(f32) + rbias[:, None, None]).astype(f32)
        dev = ((t2g * B2s[:, None, None]).astype(f32) + r2s).astype(f32)
        got = np.clip(np.round(dev), -7.0, 7.0)
        if not np.array_equal(got, want):
            continue
        # tie-margin: exact value far enough from half-integers (so device
        # fma-vs-two-round differences, bounded ~6e-6 abs in-range, cannot
        # flip a round) unless the result saturates either way
        z = (t2g.astype(np.float64) * B2s.astype(np.float64)[:, None, None]
             + x1g.astype(np.float64) * float(rscale)
             + rbias.astype(np.float64)[:, None, None])
        dist = np.abs(z - (np.floor(z) + 0.5))
        safe = (dist > 3e-5) | (np.abs(z) > 7.6)
        if bool(np.all(safe)):
            return B2s, rscale, rbias, f32(s)
    raise AssertionError(f"no verified fused scale for Th2={Th2}")


def _host_prep(x, w1, w2, g1, b1, m1, v1, g2, b2, m2, v2):
    w1 = w1.astype(f32)
    w2 = w2.astype(f32)
    sw1 = np.abs(w1).mean(axis=(1, 2, 3), dtype=np.float32).astype(f32)
    sw2 = np.abs(w2).mean(axis=(1, 2, 3), dtype=np.float32).astype(f32)
    bw1, bb1, Tw1 = _bn_consts(g1, b1, m1, v1)
    bw2, bb2, Tw2 = _bn_consts(g2, b2, m2, v2)
    sc1, Th1 = _sc_th(Tw1)
    sc2, Th2 = _sc_th(Tw2)
    s1 = _pick_scale(Th1)
    B2s, rscale, rbias, _s2 = _pick_fused_stage2(bw2, bb2, sc2, Th2)

    def wtiles(w):
        sg = (np.sign(w) * 0.25).astype(np.float16)  # [O, I, 3, 3]
        t = np.empty((C, 9, C), np.float16)          # [ci, s, co]
        for s, (ky, kx) in enumerate(SHIFTS):
            t[:, s, :] = sg[:, :, ky, kx].T
        return t

    cv = np.zeros((C, 12), f32)
    cv[:, 0] = sw1                       # A1
    cv[:, 1] = bw1                       # B1
    cv[:, 2] = bb1 + f32(MAGIC)          # bb1 + C (exact: bb1 int, C int)
    cv[:, 3] = sc1                       # sc1 (broadcast)
    cv[:, 4] = s1                        # s1 (broadcast)
    cv[:, 5] = sw2                       # A2
    cv[:, 6] = B2s                       # bw2 * s2
    cv[:, 7] = rscale                    # sc2 * s2 (broadcast)
    cv[:, 8] = rbias                     # bb2 * s2
    return wtiles(w1), wtiles(w2), cv


# ---------------------------------------------------------------------------
# Device program
# ---------------------------------------------------------------------------

_prog_cache = {}


def _build_program():
    if "nc" in _prog_cache:
        return _prog_cache["nc"]
    nc = bacc.Bacc("TRN2", target_bir_lowering=False, debug=False,
                   num_devices=NCORES)
    d_x = nc.dram_tensor("xt", [C, BS, H, W], F32, kind="ExternalInput").ap()
    d_w1 = nc.dram_tensor("w1s", [C, 9, C], F16, kind="ExternalInput").ap()
    d_w2 = nc.dram_tensor("w2s", [C, 9, C], F16, kind="ExternalInput").ap()
    d_cv = nc.dram_tensor("cv", [C, 12], F32, kind="ExternalInput").ap()
    d_o = nc.dram_tensor("ot", [C, BS, H, W], I8, kind="ExternalOutput").ap()

    with tile.TileContext(nc) as tc:
        with tc.tile_pool(name="const", bufs=1) as const, \
             tc.tile_pool(name="pads", bufs=1) as pads, \
             tc.tile_pool(name="xin", bufs=2) as xin, \
             tc.tile_pool(name="tmp", bufs=1) as tmp, \
             tc.tile_pool(name="outp", bufs=1) as outp, \
             tc.tile_pool(name="psum", bufs=2, space="PSUM") as psum:

            cv = const.tile([C, 12], F32)
            nc.sync.dma_start(out=cv, in_=d_cv)
            w1 = const.tile([C, 9, C], F16)
            nc.sync.dma_start(out=w1, in_=d_w1)
            w2 = const.tile([C, 9, C], F16)
            nc.sync.dma_start(out=w2, in_=d_w2)

            A1, B1, BB1C, SC1, S1 = (cv[:, i:i + 1] for i in range(5))
            A2, B2S, RSC, RBI = (cv[:, i:i + 1] for i in range(5, 9))

            xh = pads.tile([C, BS, HP, WP], F16)
            xl = pads.tile([C, BS, HP, WP], F16)
            x1p = pads.tile([C, BS, HP, WP], F16)
            for buf in (xh, xl, x1p):
                for b in range(BS):
                    nc.gpsimd.memset(buf[:, b, 0, :], 0.0)
                    nc.gpsimd.memset(buf[:, b, HP - 1, :], 0.0)
                    nc.gpsimd.memset(buf[:, b, 1:HP - 1, 0], 0.0)
                    nc.gpsimd.memset(buf[:, b, 1:HP - 1, WP - 1], 0.0)

            out_sb = outp.tile([C, BS, H, W], I8)

            # ---- load + hi/lo split (per image) ----
            xfs = []
            for b in range(BS):
                xf = xin.tile([C, H, W], F32, tag="xf")
                nc.sync.dma_start(out=xf, in_=d_x[:, b])
                # hi on ACT (any nearest rounding ok: lo compensates exactly)
                nc.scalar.activation(out=xh[:, b, 1:1 + H, 1:1 + W],
                                     in_=xf[:], func=IDENT)
                nc.vector.scalar_tensor_tensor(
                    out=xl[:, b, 1:1 + H, 1:1 + W], in0=xf[:], scalar=1.0,
                    in1=xh[:, b, 1:1 + H, 1:1 + W],
                    op0=Op.mult, op1=Op.subtract)
                xfs.append(xf)

            def conv(dst_y, wt, src_pad, b, two_pass):
                """9-shift conv into psum groups; RNE(psum) -> dst_y (i32)."""
                for g0, gn in GROUPS:
                    ps = psum.tile([C, 4, BANK], F32, tag="ps")
                    for k in range(gn):
                        r0 = (g0 + k) * HB
                        for s, (ky, kx) in enumerate(SHIFTS):
                            rh = src_pad[0][:, b, r0 + ky:r0 + ky + HB,
                                            kx:kx + W]
                            nc.tensor.matmul(
                                ps[:, k, 0:CHUNK], wt[:, s, :], rh,
                                start=(s == 0), stop=(s == 8 and not two_pass))
                            if two_pass:
                                rl = src_pad[1][:, b, r0 + ky:r0 + ky + HB,
                                                kx:kx + W]
                                nc.tensor.matmul(
                                    ps[:, k, 0:CHUNK], wt[:, s, :], rl,
                                    start=False, stop=(s == 8))
                    # y slab: RNE cast of the whole group in one op
                    nc.vector.tensor_scalar(
                        out=dst_y[:, g0 * CHUNK:(g0 + gn) * CHUNK],
                        in0=ps[:, 0:gn, 0:CHUNK], scalar1=1.0, scalar2=None,
                        op0=Op.mult)

            # ---- stage 1 ----
            for b in range(BS):
                y = tmp.tile([C, H * W], I32, tag="y", bufs=2)
                conv(y, w1, (xh, xl), b, two_pass=True)
                # t = floor(y*sw1) = RNE(y*sw1 - 0.5)
                t = tmp.tile([C, H * W], I32, tag="t")
                nc.vector.tensor_scalar(out=t, in0=y[:], scalar1=A1,
                                        scalar2=0.5, op0=Op.mult,
                                        op1=Op.subtract)
                # ra = RNE(x*sc1 + bb1) + MAGIC   (ACT; fp32 add at ulp=1)
                ra = tmp.tile([C, H * W], F32, tag="ra")
                nc.scalar.activation(out=ra, in_=xfs[b][:], func=IDENT,
                                     bias=BB1C, scale=SC1)
                # v' = t*bw1 + ra  (= v + MAGIC, exact ints)
                vp = tmp.tile([C, H * W], F32, tag="vp")
                nc.vector.scalar_tensor_tensor(out=vp, in0=t[:], scalar=B1,
                                               in1=ra[:], op0=Op.mult,
                                               op1=Op.add)
                # u = RNE((v' - MAGIC)*s1)
                u = tmp.tile([C, H * W], I32, tag="u")
                nc.vector.tensor_scalar(out=u, in0=vp[:], scalar1=MAGIC,
                                        scalar2=S1, op0=Op.subtract,
                                        op1=Op.mult)
                # x1 = clip(u,-7,7) -> fp16 into padded buffer
                nc.vector.tensor_scalar(
                    out=x1p[:, b, 1:1 + H, 1:1 + W], in0=u[:],
                    scalar1=7.0, scalar2=-7.0, op0=Op.min, op1=Op.max)

            # ---- stage 2 ----
            for b in range(BS):
                y2 = tmp.tile([C, H * W], I32, tag="y", bufs=2)
                conv(y2, w2, (x1p,), b, two_pass=False)
                t2 = tmp.tile([C, H * W], I32, tag="t")
                nc.vector.tensor_scalar(out=t2, in0=y2[:], scalar1=A2,
                                        scalar2=0.5, op0=Op.mult,
                                        op1=Op.subtract)
                # r2s = x1*(sc2*s2) + bb2*s2   (ACT)
                r2s = tmp.tile([C, H * W], F32, tag="ra")
                nc.scalar.activation(out=r2s,
                                     in_=x1p[:, b, 1:1 + H, 1:1 + W],
                                     func=IDENT, bias=RBI, scale=RSC)
                # u2 = RNE(t2*(bw2*s2) + r2s)   (fused, host-verified)
                u2 = tmp.tile([C, H * W], I32, tag="vp")
                nc.vector.scalar_tensor_tensor(out=u2, in0=t2[:], scalar=B2S,
                                               in1=r2s[:], op0=Op.mult,
                                               op1=Op.add)
                nc.vector.tensor_scalar(
                    out=out_sb[:, b], in0=u2[:],
                    scalar1=7.0, scalar2=-7.0, op0=Op.min, op1=Op.max)

            nc.sync.dma_start(out=d_o, in_=out_sb[:])

    nc.compile()
    _prog_cache["nc"] = nc
    return nc


# ---------------------------------------------------------------------------
# Entry point
# ---------------------------------------------------------------------------

last_results = None


def kernel(x, w1, w2, gamma1, beta1, mean1, var1,
           gamma2, beta2, mean2, var2):
    global last_results
    w1t, w2t, cv = _host_prep(x, w1, w2, gamma1, beta1, mean1, var1,
                              gamma2, beta2, mean2, var2)
    nc = _build_program()

    in_maps = []
    for i in range(NCORES):
        shard = np.ascontiguousarray(
            x[i * BS:(i + 1) * BS].astype(f32).transpose(1, 0, 2, 3))
        in_maps.append({"xt": shard, "w1s": w1t, "w2s": w2t, "cv": cv})

    trace = bool(int(os.environ.get("KERNEL_TRACE", "0")))
    kwargs = {}
    if trace:
        import concourse.bass_utils as _bu
        _bu.upload_artifacts = lambda tmpdir: ""
        kwargs["tmpdir"] = os.environ.get("KERNEL_TRACE_DIR", "/tmp/ktrace")
        os.makedirs(kwargs["tmpdir"], exist_ok=True)
    res = run_bass_kernel_spmd(nc, in_maps, core_ids=list(range(NCORES)),
                               trace=trace, **kwargs)
    last_results = res

    out = np.empty((B, C, H, W), np.float32)
    for i in range(NCORES):
        out[i * BS:(i + 1) * BS] = \
            res.results[i]["ot"].astype(np.float32).transpose(1, 0, 2, 3)
    return out


# revision 7
# speedup vs baseline: 1.0931x; 1.0831x over previous
"""Trainium2 Bass kernel for nn_BasicBlock_1w8a_q (quantized ResNet BasicBlock,
1-bit weights / 8-bit activations).

Strategy:
 - Pure data parallel over 8 NeuronCores: batch 32 -> 4 images per core.
 - Layout: channels C=128 on SBUF partitions, spatial on the free dim.
 - Each 3x3 conv with sign(+-1) weights = 9 shifted matmuls accumulating in
   PSUM (lhsT = [C_in, C_out] weight slice, rhs = shifted padded input view).
 - /4 is folded into the weights (+-0.25, exact in fp16; power-of-2 scaling
   commutes with IEEE rounding so psum == conv/4 bit-for-bit).
 - conv1 runs in 2 fp16 passes (x = hi + lo split, ~f32-exact);
   conv2 runs in 1 fp16 pass (x1 is integers in [-7,7]: exact).
 - Rounds use the DVE f32->int32 cast (exact round-half-to-even, verified on
   HW, == jnp.round) or the +1.5*2^23 magic-constant trick (fp32 add rounds
   to integer by RNE), which lets the ScalarE (ACT) do rounds too.
 - BN folding / per-channel constants are computed on host mirroring the
   reference's f32 op order; data-dependent fused scales are grid-verified
   on host against the reference mapping before use.
"""

import os

import numpy as np

import concourse.bacc as bacc
import concourse.tile as tile
import concourse.mybir as mybir
from concourse.bass_utils import run_bass_kernel_spmd
from concourse.mybir import AluOpType as Op

F32 = mybir.dt.float32
F16 = mybir.dt.float16
I32 = mybir.dt.int32
I8 = mybir.dt.int8
IDENT = mybir.ActivationFunctionType.Identity

B, C, H, W = 32, 128, 56, 56
NCORES = 8
BS = B // NCORES            # images per core
HP, WP = H + 2, W + 2       # padded spatial
HB = 8                      # output rows per psum chunk
NCH = H // HB               # chunks per image (7)
CHUNK = HB * W              # 448 columns per psum chunk
BANK = 512                  # fp32 slots per PSUM bank
GROUPS = [(0, 4), (4, 3)]   # (first chunk, n chunks) per psum group
SHIFTS = [(ky, kx) for ky in range(3) for kx in range(3)]
MAGIC = float(np.float32(12582912.0))   # 1.5 * 2^23, even integer

f32 = np.float32


# ---------------------------------------------------------------------------
# Host-side prep: mirrors the reference's f32 op order exactly.
# ---------------------------------------------------------------------------

def _qfn(x, prec):
    n = f32(2.0 ** prec - 1.0)
    q = (np.round(x * n) / n).astype(f32)
    return (x + (q - x)).astype(f32)


def _my_quantize(x, prec):
    T = np.clip(np.max(np.abs(x)), f32(1e-10), f32(255.0)).astype(f32)
    return (_qfn((np.clip(x, -T, T) / T).astype(f32), prec) * T).astype(f32)


def _bn_consts(gamma, beta, mean, var):
    gamma, beta, mean, var = (a.astype(f32) for a in (gamma, beta, mean, var))
    std = np.sqrt(var + f32(1e-5)).astype(f32)
    w = (gamma / std).astype(f32)
    bq = (beta - w * mean).astype(f32)
    T_w = np.max(np.abs(w)).astype(f32)
    bw = (_qfn((np.clip(w, -T_w, T_w) / T_w).astype(f32), 3) * f32(7.0)).astype(f32)
    qb = _my_quantize(bq, 14)
    t = (qb * f32(7.0)).astype(f32)
    t = (t * f32(1023.0)).astype(f32)
    t = (t / f32(4032.0)).astype(f32)
    t = (t * f32(7.0)).astype(f32)
    t = (t / T_w).astype(f32)
    bb = np.round(t).astype(f32)
    return bw, bb, T_w


def _sc_th(T_w):
    a = (f32(1023.0) / f32(4032.0)).astype(f32)
    a = (a * f32(7.0)).astype(f32)
    sc = np.round((a / T_w).astype(f32)).astype(f32)
    b2 = (f32(7.0) * f32(1023.0)).astype(f32)
    b2 = (b2 / f32(4032.0)).astype(f32)
    b2 = (b2 * f32(7.0)).astype(f32)
    Th = np.round((b2 / T_w).astype(f32)).astype(f32)
    return sc, Th


def _ref_final_vec(k, Th):
    # reference: round(clip(k,-Th,Th)/Th*7.0) elementwise in f32
    kk = np.clip(k.astype(f32), -Th, Th).astype(f32)
    return np.round(((kk / Th).astype(f32) * f32(7.0)).astype(f32))


def _scale_cands(Th):
    base = f32(f32(7.0) / f32(Th))
    out = [base]
    up, dn = base, base
    for _ in range(8):
        up = np.nextafter(up, f32(np.inf), dtype=f32)
        dn = np.nextafter(dn, f32(-np.inf), dtype=f32)
        out += [up, dn]
    return out


def _pick_scale(Th):
    """s (f32) with clip(RNE(k*s),-7,7) == round(clip(k,-Th,Th)/Th*7) for all
    integer k (device RNE == np.round, verified on HW)."""
    kk = np.arange(-3000, 3001, dtype=f32)
    want = _ref_final_vec(kk, f32(Th))
    for s in _scale_cands(Th):
        got = np.clip(np.round((kk * s).astype(f32)), -7.0, 7.0)
        if np.array_equal(got, want):
            return f32(s)
    raise AssertionError(f"no matching scale for Th={Th}")


def _pick_fused_stage2(bw2, bb2, sc2, Th2):
    """Stage-2 fusion: u2 = RNE(t2*(bw2*s) + (x1*(sc2*s) + bb2*s)) must equal
    ref round(clip(v2)/Th2*7) (then clip +-7) for v2 = t2*bw2 + x1*sc2 + bb2.
    Returns (B2s, rscale, rbias, s) all f32, host-verified over a full grid
    with a tie-margin so ACT fma-vs-two-round ambiguity cannot flip a round.
    """
    t2g = np.arange(-640, 641, dtype=f32)[None, :, None]       # [1,T,1]
    x1g = np.arange(-7, 8, dtype=f32)[None, None, :]           # [1,1,15]
    bwc = bw2.astype(f32)[:, None, None]                       # [C,1,1]
    bbc = bb2.astype(f32)[:, None, None]
    v2 = (t2g * bwc + x1g * f32(sc2) + bbc).astype(f32)        # exact ints
    want = np.clip(_ref_final_vec(v2, f32(Th2)), -7.0, 7.0)
    base = f32(f32(7.0) / f32(Th2))
    for j in range(0, 60):
        s = f32(base * f32(1.0 + j * 2.0 ** -19))
        B2s = (bw2 * s).astype(f32)
        rscale = f32(f32(sc2) * s)
        rbias = (bb2 * s).astype(f32)
        # device sim (two-round form)
        r2s = ((x1g * rscale).astype(f32) + rbias[:, None, None]).astype(f32)
        dev = ((t2g * B2s[:, None, None]).astype(f32) + r2s).astype(f32)
        got = np.clip(np.round(dev), -7.0, 7.0)
        if not np.array_equal(got, want):
            continue
        # tie-margin: exact value far enough from half-integers (so device
        # fma-vs-two-round differences, bounded ~6e-6 abs in-range, cannot
        # flip a round) unless the result saturates either way
        z = (t2g.astype(np.float64) * B2s.astype(np.float64)[:, None, None]
             + x1g.astype(np.float64) * float(rscale)
             + rbias.astype(np.float64)[:, None, None])
        dist = np.abs(z - (np.floor(z) + 0.5))
        safe = (dist > 3e-5) | (np.abs(z) > 7.6)
        if bool(np.all(safe)):
            return B2s, rscale, rbias, f32(s)
    raise AssertionError(f"no verified fused scale for Th2={Th2}")


def _host_prep(x, w1, w2, g1, b1, m1, v1, g2, b2, m2, v2):
    w1 = w1.astype(f32)
    w2 = w2.astype(f32)
    sw1 = np.abs(w1).mean(axis=(1, 2, 3), dtype=np.float32).astype(f32)
    sw2 = np.abs(w2).mean(axis=(1, 2, 3), dtype=np.float32).astype(f32)
    bw1, bb1, Tw1 = _bn_consts(g1, b1, m1, v1)
    bw2, bb2, Tw2 = _bn_consts(g2, b2, m2, v2)
    sc1, Th1 = _sc_th(Tw1)
    sc2, Th2 = _sc_th(Tw2)
    s1 = _pick_scale(Th1)
    B2s, rscale, rbias, _s2 = _pick_fused_stage2(bw2, bb2, sc2, Th2)

    def wtiles(w):
        sg = (np.sign(w) * 0.25).astype(np.float16)  # [O, I, 3, 3]
        t = np.empty((C, 9, C), np.float16)          # [ci, s, co]
        for s, (ky, kx) in enumerate(SHIFTS):
            t[:, s, :] = sg[:, :, ky, kx].T
        return t

    cv = np.zeros((C, 12), f32)
    cv[:, 0] = sw1                       # A1
    cv[:, 1] = bw1                       # B1
    cv[:, 2] = bb1 + f32(MAGIC)          # bb1 + C (exact: bb1 int, C int)
    cv[:, 3] = sc1                       # sc1 (broadcast)
    cv[:, 4] = s1                        # s1 (broadcast)
    cv[:, 5] = sw2                       # A2
    cv[:, 6] = B2s                       # bw2 * s2
    cv[:, 7] = rscale                    # sc2 * s2 (broadcast)
    cv[:, 8] = rbias                     # bb2 * s2
    return wtiles(w1), wtiles(w2), cv


# ---------------------------------------------------------------------------
# Device program
# ---------------------------------------------------------------------------

_prog_cache = {}


def _build_program():
    passes = int(os.environ.get("CONV1_PASSES", "2"))
    key = ("nc", passes)
    if key in _prog_cache:
        return _prog_cache[key]
    nc = bacc.Bacc("TRN2", target_bir_lowering=False, debug=False,
                   num_devices=NCORES)
    d_x = nc.dram_tensor("xt", [C, BS, H, W], F32, kind="ExternalInput").ap()
    d_w1 = nc.dram_tensor("w1s", [C, 9, C], F16, kind="ExternalInput").ap()
    d_w2 = nc.dram_tensor("w2s", [C, 9, C], F16, kind="ExternalInput").ap()
    d_cv = nc.dram_tensor("cv", [C, 12], F32, kind="ExternalInput").ap()
    d_o = nc.dram_tensor("ot", [C, BS, H, W], I8, kind="ExternalOutput").ap()

    with tile.TileContext(nc) as tc:
        with tc.tile_pool(name="const", bufs=1) as const, \
             tc.tile_pool(name="pads", bufs=1) as pads, \
             tc.tile_pool(name="xin", bufs=2) as xin, \
             tc.tile_pool(name="tmp", bufs=1) as tmp, \
             tc.tile_pool(name="outp", bufs=1) as outp, \
             tc.tile_pool(name="psum", bufs=2, space="PSUM") as psum:

            cv = const.tile([C, 12], F32)
            w1 = const.tile([C, 9, C], F16)
            w2 = const.tile([C, 9, C], F16)

            A1, B1, BB1C, SC1, S1 = (cv[:, i:i + 1] for i in range(5))
            A2, B2S, RSC, RBI = (cv[:, i:i + 1] for i in range(5, 9))

            two_pass1 = (passes == 2)
            xh = pads.tile([C, BS, HP, WP], F16)
            xl = None
            if two_pass1:
                xl = pads.tile([C, BS, HP, WP], F16)
            x1p = pads.tile([C, BS, HP, WP], F16)
            for buf in ((xh, xl, x1p) if two_pass1 else (xh, x1p)):
                for b in range(BS):
                    nc.gpsimd.memset(buf[:, b, 0, :], 0.0)
                    nc.gpsimd.memset(buf[:, b, HP - 1, :], 0.0)
                    nc.gpsimd.memset(buf[:, b, 1:HP - 1, 0], 0.0)
                    nc.gpsimd.memset(buf[:, b, 1:HP - 1, WP - 1], 0.0)

            out_sb = outp.tile([C, BS, H, W], I8)

            # ---- load + hi/lo split (two row-pieces per image, so the
            # first matmuls start as soon as piece 1 of image 0 lands) ----
            PIECES = ((0, 34), (34, H))
            xfs = []
            for b in range(BS):
                xf = xin.tile([C, H, W], F32, tag="xf")
                for r0, r1 in PIECES:
                    nc.sync.dma_start(out=xf[:, r0:r1, :],
                                      in_=d_x[:, b, r0:r1, :])
                if b == 0:
                    nc.sync.dma_start(out=cv, in_=d_cv)
                    nc.sync.dma_start(out=w1, in_=d_w1)
                    nc.sync.dma_start(out=w2, in_=d_w2)
                for r0, r1 in PIECES:
                    # hi on ACT (any nearest rounding: lo compensates exactly)
                    nc.scalar.activation(
                        out=xh[:, b, 1 + r0:1 + r1, 1:1 + W],
                        in_=xf[:, r0:r1, :], func=IDENT)
                    if two_pass1:
                        nc.vector.scalar_tensor_tensor(
                            out=xl[:, b, 1 + r0:1 + r1, 1:1 + W],
                            in0=xf[:, r0:r1, :], scalar=1.0,
                            in1=xh[:, b, 1 + r0:1 + r1, 1:1 + W],
                            op0=Op.mult, op1=Op.subtract)
                xfs.append(xf)

            def conv_group(dst_y, wt, src_pad, b, two_pass, g0, gn):
                """9-shift conv of one psum group; RNE(psum) -> dst_y slab."""
                ps = psum.tile([C, 4, BANK], F32, tag="ps")
                for k in range(gn):
                    r0 = (g0 + k) * HB
                    for s, (ky, kx) in enumerate(SHIFTS):
                        rh = src_pad[0][:, b, r0 + ky:r0 + ky + HB,
                                        kx:kx + W]
                        nc.tensor.matmul(
                            ps[:, k, 0:CHUNK], wt[:, s, :], rh,
                            start=(s == 0), stop=(s == 8 and not two_pass))
                        if two_pass:
                            rl = src_pad[1][:, b, r0 + ky:r0 + ky + HB,
                                            kx:kx + W]
                            nc.tensor.matmul(
                                ps[:, k, 0:CHUNK], wt[:, s, :], rl,
                                start=False, stop=(s == 8))
                # y slab: RNE cast of the whole group in one op
                nc.vector.tensor_scalar(
                    out=dst_y[:, g0 * CHUNK:(g0 + gn) * CHUNK],
                    in0=ps[:, 0:gn, 0:CHUNK], scalar1=1.0, scalar2=None,
                    op0=Op.mult)

            def conv(dst_y, wt, src_pad, b, two_pass):
                for g0, gn in GROUPS:
                    conv_group(dst_y, wt, src_pad, b, two_pass, g0, gn)

            # ---- stage 1 ----
            for b in range(BS):
                y = tmp.tile([C, H * W], I32, tag="y", bufs=2)
                conv(y, w1, (xh, xl), b, two_pass=two_pass1)
                # t = floor(y*sw1) = RNE(y*sw1 - 0.5)
                t = tmp.tile([C, H * W], I32, tag="t")
                nc.vector.tensor_scalar(out=t, in0=y[:], scalar1=A1,
                                        scalar2=0.5, op0=Op.mult,
                                        op1=Op.subtract)
                # ra = RNE(x*sc1 + bb1) + MAGIC   (ACT; fp32 add at ulp=1)
                ra = tmp.tile([C, H * W], F32, tag="ra")
                nc.scalar.activation(out=ra, in_=xfs[b][:], func=IDENT,
                                     bias=BB1C, scale=SC1)
                # v' = t*bw1 + ra  (= v + MAGIC, exact ints)
                vp = tmp.tile([C, H * W], F32, tag="vp")
                nc.vector.scalar_tensor_tensor(out=vp, in0=t[:], scalar=B1,
                                               in1=ra[:], op0=Op.mult,
                                               op1=Op.add)
                # u = RNE((v' - MAGIC)*s1)
                u = tmp.tile([C, H * W], I32, tag="u")
                nc.vector.tensor_scalar(out=u, in0=vp[:], scalar1=MAGIC,
                                        scalar2=S1, op0=Op.subtract,
                                        op1=Op.mult)
                # x1 = clip(u,-7,7) -> fp16 into padded buffer
                nc.vector.tensor_scalar(
                    out=x1p[:, b, 1:1 + H, 1:1 + W], in0=u[:],
                    scalar1=7.0, scalar2=-7.0, op0=Op.min, op1=Op.max)

            # ---- stage 2 (per-group elementwise to shorten the tail) ----
            for b in range(BS):
                y2 = tmp.tile([C, H * W], I32, tag="y", bufs=2)
                t2 = tmp.tile([C, H * W], I32, tag="t")
                r2s = tmp.tile([C, H * W], F32, tag="ra")
                u2 = tmp.tile([C, H * W], I32, tag="vp")
                for g0, gn in GROUPS:
                    c0, c1 = g0 * CHUNK, (g0 + gn) * CHUNK
                    rr0, rr1 = g0 * HB, (g0 + gn) * HB
                    conv_group(y2, w2, (x1p,), b, False, g0, gn)
                    nc.vector.tensor_scalar(
                        out=t2[:, c0:c1], in0=y2[:, c0:c1], scalar1=A2,
                        scalar2=0.5, op0=Op.mult, op1=Op.subtract)
                    # r2s = x1*(sc2*s2) + bb2*s2   (ACT)
                    nc.scalar.activation(
                        out=r2s[:, c0:c1],
                        in_=x1p[:, b, 1 + rr0:1 + rr1, 1:1 + W],
                        func=IDENT, bias=RBI, scale=RSC)
                    # u2 = RNE(t2*(bw2*s2) + r2s)  (fused, host-verified)
                    nc.vector.scalar_tensor_tensor(
                        out=u2[:, c0:c1], in0=t2[:, c0:c1], scalar=B2S,
                        in1=r2s[:, c0:c1], op0=Op.mult, op1=Op.add)
                    nc.vector.tensor_scalar(
                        out=out_sb[:, b, rr0:rr1, :], in0=u2[:, c0:c1],
                        scalar1=7.0, scalar2=-7.0, op0=Op.min, op1=Op.max)
                nc.sync.dma_start(out=d_o[:, b], in_=out_sb[:, b])

    nc.compile()
    _prog_cache[key] = nc
    return nc


# ---------------------------------------------------------------------------
# Entry point
# ---------------------------------------------------------------------------

last_results = None


def kernel(x, w1, w2, gamma1, beta1, mean1, var1,
           gamma2, beta2, mean2, var2):
    global last_results
    w1t, w2t, cv = _host_prep(x, w1, w2, gamma1, beta1, mean1, var1,
                              gamma2, beta2, mean2, var2)
    nc = _build_program()

    in_maps = []
    for i in range(NCORES):
        shard = np.ascontiguousarray(
            x[i * BS:(i + 1) * BS].astype(f32).transpose(1, 0, 2, 3))
        in_maps.append({"xt": shard, "w1s": w1t, "w2s": w2t, "cv": cv})

    trace = bool(int(os.environ.get("KERNEL_TRACE", "0")))
    kwargs = {}
    if trace:
        import concourse.bass_utils as _bu
        _bu.upload_artifacts = lambda tmpdir: ""
        kwargs["tmpdir"] = os.environ.get("KERNEL_TRACE_DIR", "/tmp/ktrace")
        os.makedirs(kwargs["tmpdir"], exist_ok=True)
    res = run_bass_kernel_spmd(nc, in_maps, core_ids=list(range(NCORES)),
                               trace=trace, **kwargs)
    last_results = res

    out = np.empty((B, C, H, W), np.float32)
    for i in range(NCORES):
        out[i * BS:(i + 1) * BS] = \
            res.results[i]["ot"].astype(np.float32).transpose(1, 0, 2, 3)
    return out


# revision 8
# speedup vs baseline: 1.1105x; 1.0160x over previous
"""Trainium2 Bass kernel for nn_BasicBlock_1w8a_q (quantized ResNet BasicBlock,
1-bit weights / 8-bit activations).

Strategy:
 - Pure data parallel over 8 NeuronCores: batch 32 -> 4 images per core.
 - Layout: channels C=128 on SBUF partitions, spatial on the free dim.
 - Each 3x3 conv with sign(+-1) weights = 9 shifted matmuls accumulating in
   PSUM (lhsT = [C_in, C_out] weight slice, rhs = shifted padded input view).
 - /4 is folded into the weights (+-0.25, exact in fp16; power-of-2 scaling
   commutes with IEEE rounding so psum == conv/4 bit-for-bit).
 - conv1 runs in 2 fp16 passes (x = hi + lo split, ~f32-exact);
   conv2 runs in 1 fp16 pass (x1 is integers in [-7,7]: exact).
 - Rounds use the DVE f32->int32 cast (exact round-half-to-even, verified on
   HW, == jnp.round) or the +1.5*2^23 magic-constant trick (fp32 add rounds
   to integer by RNE), which lets the ScalarE (ACT) do rounds too.
 - BN folding / per-channel constants are computed on host mirroring the
   reference's f32 op order; data-dependent fused scales are grid-verified
   on host against the reference mapping before use.
"""

import os

import numpy as np

import concourse.bass as bass
import concourse.bacc as bacc
import concourse.tile as tile
import concourse.mybir as mybir
from concourse.bass_utils import run_bass_kernel_spmd
from concourse.mybir import AluOpType as Op

F32 = mybir.dt.float32
F16 = mybir.dt.float16
I32 = mybir.dt.int32
I8 = mybir.dt.int8
F8 = mybir.dt.float8e4
WP8 = 64                    # fp8 x1 row pitch (pair stride must be %16==0)
IDENT = mybir.ActivationFunctionType.Identity

B, C, H, W = 32, 128, 56, 56
NCORES = 8
BS = B // NCORES            # images per core
HP, WP = H + 2, W + 2       # padded spatial
HB = 8                      # output rows per psum chunk
NCH = H // HB               # chunks per image (7)
CHUNK = HB * W              # 448 columns per psum chunk
BANK = 512                  # fp32 slots per PSUM bank
GROUPS = [(0, 4), (4, 3)]   # (first chunk, n chunks) per psum group
SHIFTS = [(ky, kx) for ky in range(3) for kx in range(3)]
MAGIC = float(np.float32(12582912.0))   # 1.5 * 2^23, even integer

f32 = np.float32


# ---------------------------------------------------------------------------
# Host-side prep: mirrors the reference's f32 op order exactly.
# ---------------------------------------------------------------------------

def _qfn(x, prec):
    n = f32(2.0 ** prec - 1.0)
    q = (np.round(x * n) / n).astype(f32)
    return (x + (q - x)).astype(f32)


def _my_quantize(x, prec):
    T = np.clip(np.max(np.abs(x)), f32(1e-10), f32(255.0)).astype(f32)
    return (_qfn((np.clip(x, -T, T) / T).astype(f32), prec) * T).astype(f32)


def _bn_consts(gamma, beta, mean, var):
    gamma, beta, mean, var = (a.astype(f32) for a in (gamma, beta, mean, var))
    std = np.sqrt(var + f32(1e-5)).astype(f32)
    w = (gamma / std).astype(f32)
    bq = (beta - w * mean).astype(f32)
    T_w = np.max(np.abs(w)).astype(f32)
    bw = (_qfn((np.clip(w, -T_w, T_w) / T_w).astype(f32), 3) * f32(7.0)).astype(f32)
    qb = _my_quantize(bq, 14)
    t = (qb * f32(7.0)).astype(f32)
    t = (t * f32(1023.0)).astype(f32)
    t = (t / f32(4032.0)).astype(f32)
    t = (t * f32(7.0)).astype(f32)
    t = (t / T_w).astype(f32)
    bb = np.round(t).astype(f32)
    return bw, bb, T_w


def _sc_th(T_w):
    a = (f32(1023.0) / f32(4032.0)).astype(f32)
    a = (a * f32(7.0)).astype(f32)
    sc = np.round((a / T_w).astype(f32)).astype(f32)
    b2 = (f32(7.0) * f32(1023.0)).astype(f32)
    b2 = (b2 / f32(4032.0)).astype(f32)
    b2 = (b2 * f32(7.0)).astype(f32)
    Th = np.round((b2 / T_w).astype(f32)).astype(f32)
    return sc, Th


def _ref_final_vec(k, Th):
    # reference: round(clip(k,-Th,Th)/Th*7.0) elementwise in f32
    kk = np.clip(k.astype(f32), -Th, Th).astype(f32)
    return np.round(((kk / Th).astype(f32) * f32(7.0)).astype(f32))


def _scale_cands(Th):
    base = f32(f32(7.0) / f32(Th))
    out = [base]
    up, dn = base, base
    for _ in range(8):
        up = np.nextafter(up, f32(np.inf), dtype=f32)
        dn = np.nextafter(dn, f32(-np.inf), dtype=f32)
        out += [up, dn]
    return out


def _pick_scale(Th):
    """s (f32) with clip(RNE(k*s),-7,7) == round(clip(k,-Th,Th)/Th*7) for all
    integer k (device RNE == np.round, verified on HW)."""
    kk = np.arange(-3000, 3001, dtype=f32)
    want = _ref_final_vec(kk, f32(Th))
    for s in _scale_cands(Th):
        got = np.clip(np.round((kk * s).astype(f32)), -7.0, 7.0)
        if np.array_equal(got, want):
            return f32(s)
    raise AssertionError(f"no matching scale for Th={Th}")


def _pick_fused_stage2(bw2, bb2, sc2, Th2):
    """Stage-2 fusion: u2 = RNE(t2*(bw2*s) + (x1*(sc2*s) + bb2*s)) must equal
    ref round(clip(v2)/Th2*7) (then clip +-7) for v2 = t2*bw2 + x1*sc2 + bb2.
    Returns (B2s, rscale, rbias, s) all f32, host-verified over a full grid
    with a tie-margin so ACT fma-vs-two-round ambiguity cannot flip a round.
    """
    t2g = np.arange(-640, 641, dtype=f32)[None, :, None]       # [1,T,1]
    x1g = np.arange(-7, 8, dtype=f32)[None, None, :]           # [1,1,15]
    bwc = bw2.astype(f32)[:, None, None]                       # [C,1,1]
    bbc = bb2.astype(f32)[:, None, None]
    v2 = (t2g * bwc + x1g * f32(sc2) + bbc).astype(f32)        # exact ints
    want = np.clip(_ref_final_vec(v2, f32(Th2)), -7.0, 7.0)
    base = f32(f32(7.0) / f32(Th2))
    for j in range(0, 60):
        s = f32(base * f32(1.0 + j * 2.0 ** -19))
        B2s = (bw2 * s).astype(f32)
        rscale = f32(f32(sc2) * s)
        rbias = (bb2 * s).astype(f32)
        # device sim (two-round form)
        r2s = ((x1g * rscale).astype(f32) + rbias[:, None, None]).astype(f32)
        dev = ((t2g * B2s[:, None, None]).astype(f32) + r2s).astype(f32)
        got = np.clip(np.round(dev), -7.0, 7.0)
        if not np.array_equal(got, want):
            continue
        # tie-margin: exact value far enough from half-integers (so device
        # fma-vs-two-round differences, bounded ~6e-6 abs in-range, cannot
        # flip a round) unless the result saturates either way
        z = (t2g.astype(np.float64) * B2s.astype(np.float64)[:, None, None]
             + x1g.astype(np.float64) * float(rscale)
             + rbias.astype(np.float64)[:, None, None])
        dist = np.abs(z - (np.floor(z) + 0.5))
        safe = (dist > 3e-5) | (np.abs(z) > 7.6)
        if bool(np.all(safe)):
            return B2s, rscale, rbias, f32(s)
    raise AssertionError(f"no verified fused scale for Th2={Th2}")


def _host_prep(x, w1, w2, g1, b1, m1, v1, g2, b2, m2, v2):
    w1 = w1.astype(f32)
    w2 = w2.astype(f32)
    sw1 = np.abs(w1).mean(axis=(1, 2, 3), dtype=np.float32).astype(f32)
    sw2 = np.abs(w2).mean(axis=(1, 2, 3), dtype=np.float32).astype(f32)
    bw1, bb1, Tw1 = _bn_consts(g1, b1, m1, v1)
    bw2, bb2, Tw2 = _bn_consts(g2, b2, m2, v2)
    sc1, Th1 = _sc_th(Tw1)
    sc2, Th2 = _sc_th(Tw2)
    s1 = _pick_scale(Th1)
    B2s, rscale, rbias, _s2 = _pick_fused_stage2(bw2, bb2, sc2, Th2)

    def wtiles(w):
        sg = (np.sign(w) * 0.25).astype(np.float16)  # [O, I, 3, 3]
        t = np.empty((C, 9, C), np.float16)          # [ci, s, co]
        for s, (ky, kx) in enumerate(SHIFTS):
            t[:, s, :] = sg[:, :, ky, kx].T
        return t

    def wtiles8(w):
        np8 = mybir.dt.np(F8)
        sg = (np.sign(w) * 0.25).astype(np.float32)  # [O, I, 3, 3]
        d = np.empty((C, 3, 2, C), np.float32)       # [ci, kx, ky(0,1), co]
        r = np.empty((C, 3, C), np.float32)          # [ci, kx, co] (ky=2)
        for kx in range(3):
            d[:, kx, 0, :] = sg[:, :, 0, kx].T
            d[:, kx, 1, :] = sg[:, :, 1, kx].T
            r[:, kx, :] = sg[:, :, 2, kx].T
        return d.astype(np8), r.astype(np8)

    cv = np.zeros((C, 12), f32)
    cv[:, 0] = sw1                       # A1
    cv[:, 1] = bw1                       # B1
    cv[:, 2] = bb1 + f32(MAGIC)          # bb1 + C (exact: bb1 int, C int)
    cv[:, 3] = sc1                       # sc1 (broadcast)
    cv[:, 4] = s1                        # s1 (broadcast)
    cv[:, 5] = sw2                       # A2
    cv[:, 6] = B2s                       # bw2 * s2
    cv[:, 7] = rscale                    # sc2 * s2 (broadcast)
    cv[:, 8] = rbias                     # bb2 * s2
    w2d, w2r = wtiles8(w2)
    return wtiles(w1), w2d, w2r, cv


# ---------------------------------------------------------------------------
# Device program
# ---------------------------------------------------------------------------

_prog_cache = {}


def _build_program():
    passes = int(os.environ.get("CONV1_PASSES", "2"))
    key = ("nc", passes)
    if key in _prog_cache:
        return _prog_cache[key]
    nc = bacc.Bacc("TRN2", target_bir_lowering=False, debug=False,
                   num_devices=NCORES)
    d_x = nc.dram_tensor("xt", [C, BS, H, W], F32, kind="ExternalInput").ap()
    d_w1 = nc.dram_tensor("w1s", [C, 9, C], F16, kind="ExternalInput").ap()
    d_w2d = nc.dram_tensor("w2d", [C, 3, 2, C], F8, kind="ExternalInput").ap()
    d_w2r = nc.dram_tensor("w2r", [C, 3, C], F8, kind="ExternalInput").ap()
    d_cv = nc.dram_tensor("cv", [C, 12], F32, kind="ExternalInput").ap()
    d_o = nc.dram_tensor("ot", [C, BS, H, W], I8, kind="ExternalOutput").ap()

    with tile.TileContext(nc) as tc:
        with tc.tile_pool(name="const", bufs=1) as const, \
             tc.tile_pool(name="pads", bufs=1) as pads, \
             tc.tile_pool(name="xin", bufs=2) as xin, \
             tc.tile_pool(name="tmp", bufs=1) as tmp, \
             tc.tile_pool(name="outp", bufs=1) as outp, \
             tc.tile_pool(name="psum", bufs=2, space="PSUM") as psum:

            cv = const.tile([C, 12], F32)
            w1 = const.tile([C, 9, C], F16)
            w2d = const.tile([C, 3, 2, C], F8)
            w2r = const.tile([C, 3, C], F8)

            A1, B1, BB1C, SC1, S1 = (cv[:, i:i + 1] for i in range(5))
            A2, B2S, RSC, RBI = (cv[:, i:i + 1] for i in range(5, 9))

            two_pass1 = (passes == 2)
            xh = pads.tile([C, BS, HP, WP], F16)
            xl = None
            if two_pass1:
                xl = pads.tile([C, BS, HP, WP], F16)
            x1p = pads.tile([C, BS, HP, WP8], F8)
            for buf in ((xh, xl) if two_pass1 else (xh,)):
                for b in range(BS):
                    nc.gpsimd.memset(buf[:, b, 0, :], 0.0)
                    nc.gpsimd.memset(buf[:, b, HP - 1, :], 0.0)
                    nc.gpsimd.memset(buf[:, b, 1:HP - 1, 0], 0.0)
                    nc.gpsimd.memset(buf[:, b, 1:HP - 1, WP - 1], 0.0)
            for b in range(BS):
                nc.gpsimd.memset(x1p[:, b, 0, :], 0.0)
                nc.gpsimd.memset(x1p[:, b, HP - 1, :], 0.0)
                nc.gpsimd.memset(x1p[:, b, 1:HP - 1, 0], 0.0)
                nc.gpsimd.memset(x1p[:, b, 1:HP - 1, WP - 1:], 0.0)

            out_sb = outp.tile([C, BS, H, W], I8)

            # ---- load + hi/lo split (two row-pieces per image, so the
            # first matmuls start as soon as piece 1 of image 0 lands) ----
            PIECES = ((0, 34), (34, H))
            xfs = []
            for b in range(BS):
                xf = xin.tile([C, H, W], F32, tag="xf")
                for r0, r1 in PIECES:
                    nc.sync.dma_start(out=xf[:, r0:r1, :],
                                      in_=d_x[:, b, r0:r1, :])
                if b == 0:
                    nc.sync.dma_start(out=cv, in_=d_cv)
                    nc.sync.dma_start(out=w1, in_=d_w1)
                    nc.sync.dma_start(out=w2d, in_=d_w2d)
                    nc.sync.dma_start(out=w2r, in_=d_w2r)
                for r0, r1 in PIECES:
                    # hi on ACT (any nearest rounding: lo compensates exactly)
                    nc.scalar.activation(
                        out=xh[:, b, 1 + r0:1 + r1, 1:1 + W],
                        in_=xf[:, r0:r1, :], func=IDENT)
                    if two_pass1:
                        nc.vector.scalar_tensor_tensor(
                            out=xl[:, b, 1 + r0:1 + r1, 1:1 + W],
                            in0=xf[:, r0:r1, :], scalar=1.0,
                            in1=xh[:, b, 1 + r0:1 + r1, 1:1 + W],
                            op0=Op.mult, op1=Op.subtract)
                xfs.append(xf)

            def conv_group(dst_y, wt, src_pad, b, two_pass, g0, gn):
                """9-shift conv of one psum group; RNE(psum) -> dst_y slab."""
                ps = psum.tile([C, 4, BANK], F32, tag="ps")
                for k in range(gn):
                    r0 = (g0 + k) * HB
                    for s, (ky, kx) in enumerate(SHIFTS):
                        rh = src_pad[0][:, b, r0 + ky:r0 + ky + HB,
                                        kx:kx + W]
                        nc.tensor.matmul(
                            ps[:, k, 0:CHUNK], wt[:, s, :], rh,
                            start=(s == 0), stop=(s == 8 and not two_pass))
                        if two_pass:
                            rl = src_pad[1][:, b, r0 + ky:r0 + ky + HB,
                                            kx:kx + W]
                            nc.tensor.matmul(
                                ps[:, k, 0:CHUNK], wt[:, s, :], rl,
                                start=False, stop=(s == 8))
                # y slab: RNE cast of the whole group in one op
                nc.vector.tensor_scalar(
                    out=dst_y[:, g0 * CHUNK:(g0 + gn) * CHUNK],
                    in0=ps[:, 0:gn, 0:CHUNK], scalar1=1.0, scalar2=None,
                    op0=Op.mult)

            def conv(dst_y, wt, src_pad, b, two_pass):
                for g0, gn in GROUPS:
                    conv_group(dst_y, wt, src_pad, b, two_pass, g0, gn)

            def conv2_group(dst_y, wd, wr, src, b, g0, gn):
                """conv2: 3 DoubleRow pair-MMs (ky=0,1) + 3 regular (ky=2),
                all fp8, exact for integer x1."""
                ps = psum.tile([C, 4, BANK], F32, tag="ps")
                for k in range(gn):
                    r0 = (g0 + k) * HB
                    for kx in range(3):
                        v0 = src[:, b, r0:r0 + HB, kx:kx + W]
                        pair = bass.AP(
                            tensor=v0.tensor, offset=v0.offset,
                            ap=[v0.ap[0], [WP8, 2], [WP8, HB], [1, W]])
                        nc.tensor.matmul(
                            ps[:, k, 0:CHUNK], wd[:, kx, :, :], pair,
                            perf_mode=mybir.MatmulPerfMode.DoubleRow,
                            start=(kx == 0), stop=False)
                    for kx in range(3):
                        rr = src[:, b, r0 + 2:r0 + 2 + HB, kx:kx + W]
                        nc.tensor.matmul(
                            ps[:, k, 0:CHUNK], wr[:, kx, :], rr,
                            start=False, stop=(kx == 2))
                nc.vector.tensor_scalar(
                    out=dst_y[:, g0 * CHUNK:(g0 + gn) * CHUNK],
                    in0=ps[:, 0:gn, 0:CHUNK], scalar1=1.0, scalar2=None,
                    op0=Op.mult)

            # ---- stage 1 ----
            for b in range(BS):
                y = tmp.tile([C, H * W], I32, tag="y", bufs=2)
                conv(y, w1, (xh, xl), b, two_pass=two_pass1)
                # t = floor(y*sw1) = RNE(y*sw1 - 0.5)
                t = tmp.tile([C, H * W], I32, tag="t")
                nc.vector.tensor_scalar(out=t, in0=y[:], scalar1=A1,
                                        scalar2=0.5, op0=Op.mult,
                                        op1=Op.subtract)
                # ra = RNE(x*sc1 + bb1) + MAGIC   (ACT; fp32 add at ulp=1)
                ra = tmp.tile([C, H * W], F32, tag="ra")
                nc.scalar.activation(out=ra, in_=xfs[b][:], func=IDENT,
                                     bias=BB1C, scale=SC1)
                # v' = t*bw1 + ra  (= v + MAGIC, exact ints)
                vp = tmp.tile([C, H * W], F32, tag="vp")
                nc.vector.scalar_tensor_tensor(out=vp, in0=t[:], scalar=B1,
                                               in1=ra[:], op0=Op.mult,
                                               op1=Op.add)
                # u = RNE((v' - MAGIC)*s1)
                u = tmp.tile([C, H * W], I32, tag="u")
                nc.vector.tensor_scalar(out=u, in0=vp[:], scalar1=MAGIC,
                                        scalar2=S1, op0=Op.subtract,
                                        op1=Op.mult)
                # x1 = clip(u,-7,7) -> fp16 into padded buffer
                nc.vector.tensor_scalar(
                    out=x1p[:, b, 1:1 + H, 1:1 + W], in0=u[:],
                    scalar1=7.0, scalar2=-7.0, op0=Op.min, op1=Op.max)

            # ---- stage 2 (per-group elementwise to shorten the tail) ----
            for b in range(BS):
                y2 = tmp.tile([C, H * W], I32, tag="y", bufs=2)
                t2 = tmp.tile([C, H * W], I32, tag="t")
                r2s = tmp.tile([C, H * W], F32, tag="ra")
                u2 = tmp.tile([C, H * W], I32, tag="vp")
                for g0, gn in GROUPS:
                    c0, c1 = g0 * CHUNK, (g0 + gn) * CHUNK
                    rr0, rr1 = g0 * HB, (g0 + gn) * HB
                    conv2_group(y2, w2d, w2r, x1p, b, g0, gn)
                    nc.vector.tensor_scalar(
                        out=t2[:, c0:c1], in0=y2[:, c0:c1], scalar1=A2,
                        scalar2=0.5, op0=Op.mult, op1=Op.subtract)
                    # r2s = x1*(sc2*s2) + bb2*s2   (ACT)
                    nc.scalar.activation(
                        out=r2s[:, c0:c1],
                        in_=x1p[:, b, 1 + rr0:1 + rr1, 1:1 + W],
                        func=IDENT, bias=RBI, scale=RSC)
                    # u2 = RNE(t2*(bw2*s2) + r2s)  (fused, host-verified)
                    nc.vector.scalar_tensor_tensor(
                        out=u2[:, c0:c1], in0=t2[:, c0:c1], scalar=B2S,
                        in1=r2s[:, c0:c1], op0=Op.mult, op1=Op.add)
                    nc.vector.tensor_scalar(
                        out=out_sb[:, b, rr0:rr1, :], in0=u2[:, c0:c1],
                        scalar1=7.0, scalar2=-7.0, op0=Op.min, op1=Op.max)
                nc.sync.dma_start(out=d_o[:, b], in_=out_sb[:, b])

    nc.compile()
    _prog_cache[key] = nc
    return nc


# ---------------------------------------------------------------------------
# Entry point
# ---------------------------------------------------------------------------

last_results = None


def kernel(x, w1, w2, gamma1, beta1, mean1, var1,
           gamma2, beta2, mean2, var2):
    global last_results
    w1t, w2d, w2r, cv = _host_prep(x, w1, w2, gamma1, beta1, mean1, var1,
                              gamma2, beta2, mean2, var2)
    nc = _build_program()

    in_maps = []
    for i in range(NCORES):
        shard = np.ascontiguousarray(
            x[i * BS:(i + 1) * BS].astype(f32).transpose(1, 0, 2, 3))
        in_maps.append({"xt": shard, "w1s": w1t, "w2d": w2d,
                        "w2r": w2r, "cv": cv})

    trace = bool(int(os.environ.get("KERNEL_TRACE", "0")))
    kwargs = {}
    if trace:
        import concourse.bass_utils as _bu
        _bu.upload_artifacts = lambda tmpdir: ""
        kwargs["tmpdir"] = os.environ.get("KERNEL_TRACE_DIR", "/tmp/ktrace")
        os.makedirs(kwargs["tmpdir"], exist_ok=True)
    res = run_bass_kernel_spmd(nc, in_maps, core_ids=list(range(NCORES)),
                               trace=trace, **kwargs)
    last_results = res

    out = np.empty((B, C, H, W), np.float32)
    for i in range(NCORES):
        out[i * BS:(i + 1) * BS] = \
            res.results[i]["ot"].astype(np.float32).transpose(1, 0, 2, 3)
    return out


# revision 9
# speedup vs baseline: 1.1266x; 1.0145x over previous
"""Trainium2 Bass kernel for nn_BasicBlock_1w8a_q (quantized ResNet BasicBlock,
1-bit weights / 8-bit activations).

Strategy:
 - Pure data parallel over 8 NeuronCores: batch 32 -> 4 images per core.
 - Layout: channels C=128 on SBUF partitions, spatial on the free dim.
 - Each 3x3 conv with sign(+-1) weights = 9 shifted matmuls accumulating in
   PSUM (lhsT = [C_in, C_out] weight slice, rhs = shifted padded input view).
 - /4 is folded into the weights (+-0.25, exact in fp16; power-of-2 scaling
   commutes with IEEE rounding so psum == conv/4 bit-for-bit).
 - conv1 runs in 2 fp16 passes (x = hi + lo split, ~f32-exact);
   conv2 runs in 1 fp16 pass (x1 is integers in [-7,7]: exact).
 - Rounds use the DVE f32->int32 cast (exact round-half-to-even, verified on
   HW, == jnp.round) or the +1.5*2^23 magic-constant trick (fp32 add rounds
   to integer by RNE), which lets the ScalarE (ACT) do rounds too.
 - BN folding / per-channel constants are computed on host mirroring the
   reference's f32 op order; data-dependent fused scales are grid-verified
   on host against the reference mapping before use.
"""

import os

import numpy as np

import concourse.bass as bass
import concourse.bacc as bacc
import concourse.tile as tile
import concourse.mybir as mybir
from concourse.bass_utils import run_bass_kernel_spmd
from concourse.mybir import AluOpType as Op

F32 = mybir.dt.float32
F16 = mybir.dt.float16
I32 = mybir.dt.int32
I8 = mybir.dt.int8
F8 = mybir.dt.float8e4
WP8 = 64                    # fp8 x1 row pitch (pair stride must be %16==0)
IDENT = mybir.ActivationFunctionType.Identity

B, C, H, W = 32, 128, 56, 56
NCORES = 8
BS = B // NCORES            # images per core
HP, WP = H + 2, W + 2       # padded spatial
HB = 8                      # output rows per psum chunk
NCH = H // HB               # chunks per image (7)
CHUNK = HB * W              # 448 columns per psum chunk
BANK = 512                  # fp32 slots per PSUM bank
GROUPS = [(0, 4), (4, 3)]   # (first chunk, n chunks) per psum group
SHIFTS = [(ky, kx) for ky in range(3) for kx in range(3)]
MAGIC = float(np.float32(12582912.0))   # 1.5 * 2^23, even integer

f32 = np.float32


# ---------------------------------------------------------------------------
# Host-side prep: mirrors the reference's f32 op order exactly.
# ---------------------------------------------------------------------------

def _qfn(x, prec):
    n = f32(2.0 ** prec - 1.0)
    q = (np.round(x * n) / n).astype(f32)
    return (x + (q - x)).astype(f32)


def _my_quantize(x, prec):
    T = np.clip(np.max(np.abs(x)), f32(1e-10), f32(255.0)).astype(f32)
    return (_qfn((np.clip(x, -T, T) / T).astype(f32), prec) * T).astype(f32)


def _bn_consts(gamma, beta, mean, var):
    gamma, beta, mean, var = (a.astype(f32) for a in (gamma, beta, mean, var))
    std = np.sqrt(var + f32(1e-5)).astype(f32)
    w = (gamma / std).astype(f32)
    bq = (beta - w * mean).astype(f32)
    T_w = np.max(np.abs(w)).astype(f32)
    bw = (_qfn((np.clip(w, -T_w, T_w) / T_w).astype(f32), 3) * f32(7.0)).astype(f32)
    qb = _my_quantize(bq, 14)
    t = (qb * f32(7.0)).astype(f32)
    t = (t * f32(1023.0)).astype(f32)
    t = (t / f32(4032.0)).astype(f32)
    t = (t * f32(7.0)).astype(f32)
    t = (t / T_w).astype(f32)
    bb = np.round(t).astype(f32)
    return bw, bb, T_w


def _sc_th(T_w):
    a = (f32(1023.0) / f32(4032.0)).astype(f32)
    a = (a * f32(7.0)).astype(f32)
    sc = np.round((a / T_w).astype(f32)).astype(f32)
    b2 = (f32(7.0) * f32(1023.0)).astype(f32)
    b2 = (b2 / f32(4032.0)).astype(f32)
    b2 = (b2 * f32(7.0)).astype(f32)
    Th = np.round((b2 / T_w).astype(f32)).astype(f32)
    return sc, Th


def _ref_final_vec(k, Th):
    # reference: round(clip(k,-Th,Th)/Th*7.0) elementwise in f32
    kk = np.clip(k.astype(f32), -Th, Th).astype(f32)
    return np.round(((kk / Th).astype(f32) * f32(7.0)).astype(f32))


def _scale_cands(Th):
    base = f32(f32(7.0) / f32(Th))
    out = [base]
    up, dn = base, base
    for _ in range(8):
        up = np.nextafter(up, f32(np.inf), dtype=f32)
        dn = np.nextafter(dn, f32(-np.inf), dtype=f32)
        out += [up, dn]
    return out


def _pick_scale(Th):
    """s (f32) with clip(RNE(k*s),-7,7) == round(clip(k,-Th,Th)/Th*7) for all
    integer k (device RNE == np.round, verified on HW)."""
    kk = np.arange(-3000, 3001, dtype=f32)
    want = _ref_final_vec(kk, f32(Th))
    for s in _scale_cands(Th):
        got = np.clip(np.round((kk * s).astype(f32)), -7.0, 7.0)
        if np.array_equal(got, want):
            return f32(s)
    raise AssertionError(f"no matching scale for Th={Th}")


def _pick_fused_stage2(bw2, bb2, sc2, Th2):
    """Stage-2 fusion: u2 = RNE(t2*(bw2*s) + (x1*(sc2*s) + bb2*s)) must equal
    ref round(clip(v2)/Th2*7) (then clip +-7) for v2 = t2*bw2 + x1*sc2 + bb2.
    Returns (B2s, rscale, rbias, s) all f32, host-verified over a full grid
    with a tie-margin so ACT fma-vs-two-round ambiguity cannot flip a round.
    """
    t2g = np.arange(-640, 641, dtype=f32)[None, :, None]       # [1,T,1]
    x1g = np.arange(-7, 8, dtype=f32)[None, None, :]           # [1,1,15]
    bwc = bw2.astype(f32)[:, None, None]                       # [C,1,1]
    bbc = bb2.astype(f32)[:, None, None]
    v2 = (t2g * bwc + x1g * f32(sc2) + bbc).astype(f32)        # exact ints
    want = np.clip(_ref_final_vec(v2, f32(Th2)), -7.0, 7.0)
    base = f32(f32(7.0) / f32(Th2))
    for j in range(0, 60):
        s = f32(base * f32(1.0 + j * 2.0 ** -19))
        B2s = (bw2 * s).astype(f32)
        rscale = f32(f32(sc2) * s)
        rbias = (bb2 * s).astype(f32)
        # device sim (two-round form)
        r2s = ((x1g * rscale).astype(f32) + rbias[:, None, None]).astype(f32)
        dev = ((t2g * B2s[:, None, None]).astype(f32) + r2s).astype(f32)
        got = np.clip(np.round(dev), -7.0, 7.0)
        if not np.array_equal(got, want):
            continue
        # tie-margin: exact value far enough from half-integers (so device
        # fma-vs-two-round differences, bounded ~6e-6 abs in-range, cannot
        # flip a round) unless the result saturates either way
        z = (t2g.astype(np.float64) * B2s.astype(np.float64)[:, None, None]
             + x1g.astype(np.float64) * float(rscale)
             + rbias.astype(np.float64)[:, None, None])
        dist = np.abs(z - (np.floor(z) + 0.5))
        safe = (dist > 3e-5) | (np.abs(z) > 7.6)
        if bool(np.all(safe)):
            return B2s, rscale, rbias, f32(s)
    raise AssertionError(f"no verified fused scale for Th2={Th2}")


def _host_prep(x, w1, w2, g1, b1, m1, v1, g2, b2, m2, v2):
    w1 = w1.astype(f32)
    w2 = w2.astype(f32)
    sw1 = np.abs(w1).mean(axis=(1, 2, 3), dtype=np.float32).astype(f32)
    sw2 = np.abs(w2).mean(axis=(1, 2, 3), dtype=np.float32).astype(f32)
    bw1, bb1, Tw1 = _bn_consts(g1, b1, m1, v1)
    bw2, bb2, Tw2 = _bn_consts(g2, b2, m2, v2)
    sc1, Th1 = _sc_th(Tw1)
    sc2, Th2 = _sc_th(Tw2)
    s1 = _pick_scale(Th1)
    B2s, rscale, rbias, _s2 = _pick_fused_stage2(bw2, bb2, sc2, Th2)

    def wtiles(w):
        sg = (np.sign(w) * 0.25).astype(np.float16)  # [O, I, 3, 3]
        t = np.empty((C, 9, C), np.float16)          # [ci, s, co]
        for s, (ky, kx) in enumerate(SHIFTS):
            t[:, s, :] = sg[:, :, ky, kx].T
        return t

    def wtiles8(w):
        np8 = mybir.dt.np(F8)
        sg = (np.sign(w) * 0.25).astype(np.float32)  # [O, I, 3, 3]
        d = np.empty((C, 3, 2, C), np.float32)       # [ci, kx, ky(0,1), co]
        r = np.empty((C, 3, C), np.float32)          # [ci, kx, co] (ky=2)
        for kx in range(3):
            d[:, kx, 0, :] = sg[:, :, 0, kx].T
            d[:, kx, 1, :] = sg[:, :, 1, kx].T
            r[:, kx, :] = sg[:, :, 2, kx].T
        return d.astype(np8), r.astype(np8)

    cv = np.zeros((C, 12), f32)
    cv[:, 0] = sw1                       # A1
    cv[:, 1] = bw1                       # B1
    cv[:, 2] = bb1 + f32(MAGIC)          # bb1 + C (exact: bb1 int, C int)
    cv[:, 3] = sc1                       # sc1 (broadcast)
    cv[:, 4] = s1                        # s1 (broadcast)
    cv[:, 5] = sw2                       # A2
    cv[:, 6] = B2s                       # bw2 * s2
    cv[:, 7] = rscale                    # sc2 * s2 (broadcast)
    cv[:, 8] = rbias                     # bb2 * s2
    w2d, w2r = wtiles8(w2)
    return wtiles(w1), w2d, w2r, cv


# ---------------------------------------------------------------------------
# Device program
# ---------------------------------------------------------------------------

_prog_cache = {}


def _build_program():
    passes = int(os.environ.get("CONV1_PASSES", "2"))
    key = ("nc", passes)
    if key in _prog_cache:
        return _prog_cache[key]
    nc = bacc.Bacc("TRN2", target_bir_lowering=False, debug=False,
                   num_devices=NCORES)
    d_x = nc.dram_tensor("xt", [C, BS, H, W], F32, kind="ExternalInput").ap()
    d_w1 = nc.dram_tensor("w1s", [C, 9, C], F16, kind="ExternalInput").ap()
    d_w2d = nc.dram_tensor("w2d", [C, 3, 2, C], F8, kind="ExternalInput").ap()
    d_w2r = nc.dram_tensor("w2r", [C, 3, C], F8, kind="ExternalInput").ap()
    d_cv = nc.dram_tensor("cv", [C, 12], F32, kind="ExternalInput").ap()
    d_o = nc.dram_tensor("ot", [C, BS, H, W], I8, kind="ExternalOutput").ap()

    with tile.TileContext(nc) as tc:
        with tc.tile_pool(name="const", bufs=1) as const, \
             tc.tile_pool(name="pads", bufs=1) as pads, \
             tc.tile_pool(name="xin", bufs=2) as xin, \
             tc.tile_pool(name="tmp", bufs=1) as tmp, \
             tc.tile_pool(name="outp", bufs=1) as outp, \
             tc.tile_pool(name="psum", bufs=2, space="PSUM") as psum:

            cv = const.tile([C, 12], F32)
            w1 = const.tile([C, 9, C], F16)
            w2d = const.tile([C, 3, 2, C], F8)
            w2r = const.tile([C, 3, C], F8)

            A1, B1, BB1C, SC1, S1 = (cv[:, i:i + 1] for i in range(5))
            A2, B2S, RSC, RBI = (cv[:, i:i + 1] for i in range(5, 9))

            two_pass1 = (passes == 2)
            xh = pads.tile([C, BS, HP, WP], F16)
            xl = None
            if two_pass1:
                xl = pads.tile([C, BS, HP, WP], F16)
            x1p = pads.tile([C, BS, HP, WP8], F8)
            for buf in ((xh, xl) if two_pass1 else (xh,)):
                for b in range(BS):
                    nc.gpsimd.memset(buf[:, b, 0, :], 0.0)
                    nc.gpsimd.memset(buf[:, b, HP - 1, :], 0.0)
                    nc.gpsimd.memset(buf[:, b, 1:HP - 1, 0], 0.0)
                    nc.gpsimd.memset(buf[:, b, 1:HP - 1, WP - 1], 0.0)
            for b in range(BS):
                nc.gpsimd.memset(x1p[:, b, 0, :], 0.0)
                nc.gpsimd.memset(x1p[:, b, HP - 1, :], 0.0)
                nc.gpsimd.memset(x1p[:, b, 1:HP - 1, 0], 0.0)
                nc.gpsimd.memset(x1p[:, b, 1:HP - 1, WP - 1:], 0.0)

            out_sb = outp.tile([C, BS, H, W], I8)

            # ---- load + hi/lo split (two row-pieces per image, so the
            # first matmuls start as soon as piece 1 of image 0 lands) ----
            PIECES = ((0, 34), (34, H))
            xfs = []
            for b in range(BS):
                xf = xin.tile([C, H, W], F32, tag="xf")
                for r0, r1 in PIECES:
                    nc.sync.dma_start(out=xf[:, r0:r1, :],
                                      in_=d_x[:, b, r0:r1, :])
                if b == 0:
                    nc.sync.dma_start(out=cv, in_=d_cv)
                    nc.sync.dma_start(out=w1, in_=d_w1)
                    nc.sync.dma_start(out=w2d, in_=d_w2d)
                    nc.sync.dma_start(out=w2r, in_=d_w2r)
                for r0, r1 in PIECES:
                    # hi on ACT (any nearest rounding: lo compensates exactly)
                    nc.scalar.activation(
                        out=xh[:, b, 1 + r0:1 + r1, 1:1 + W],
                        in_=xf[:, r0:r1, :], func=IDENT)
                    if two_pass1:
                        nc.vector.scalar_tensor_tensor(
                            out=xl[:, b, 1 + r0:1 + r1, 1:1 + W],
                            in0=xf[:, r0:r1, :], scalar=1.0,
                            in1=xh[:, b, 1 + r0:1 + r1, 1:1 + W],
                            op0=Op.mult, op1=Op.subtract)
                xfs.append(xf)

            def conv_group(dst_y, wt, src_pad, b, two_pass, g0, gn):
                """9-shift conv of one psum group; RNE(psum) -> dst_y slab."""
                ps = psum.tile([C, 4, BANK], F32, tag="ps")
                for k in range(gn):
                    r0 = (g0 + k) * HB
                    for s, (ky, kx) in enumerate(SHIFTS):
                        rh = src_pad[0][:, b, r0 + ky:r0 + ky + HB,
                                        kx:kx + W]
                        nc.tensor.matmul(
                            ps[:, k, 0:CHUNK], wt[:, s, :], rh,
                            start=(s == 0), stop=(s == 8 and not two_pass))
                        if two_pass:
                            rl = src_pad[1][:, b, r0 + ky:r0 + ky + HB,
                                            kx:kx + W]
                            nc.tensor.matmul(
                                ps[:, k, 0:CHUNK], wt[:, s, :], rl,
                                start=False, stop=(s == 8))
                # y slab: RNE cast of the whole group in one op
                nc.vector.tensor_scalar(
                    out=dst_y[:, g0 * CHUNK:(g0 + gn) * CHUNK],
                    in0=ps[:, 0:gn, 0:CHUNK], scalar1=1.0, scalar2=None,
                    op0=Op.mult)

            def conv(dst_y, wt, src_pad, b, two_pass):
                for g0, gn in GROUPS:
                    conv_group(dst_y, wt, src_pad, b, two_pass, g0, gn)

            def conv2_group(dst_y, wd, wr, src, b, g0, gn):
                """conv2: 3 DoubleRow pair-MMs (ky=0,1) + 3 regular (ky=2),
                all fp8, exact for integer x1."""
                ps = psum.tile([C, 4, BANK], F32, tag="ps")
                for k in range(gn):
                    r0 = (g0 + k) * HB
                    for kx in range(3):
                        v0 = src[:, b, r0:r0 + HB, kx:kx + W]
                        pair = bass.AP(
                            tensor=v0.tensor, offset=v0.offset,
                            ap=[v0.ap[0], [WP8, 2], [WP8, HB], [1, W]])
                        nc.tensor.matmul(
                            ps[:, k, 0:CHUNK], wd[:, kx, :, :], pair,
                            perf_mode=mybir.MatmulPerfMode.DoubleRow,
                            start=(kx == 0), stop=False)
                    for kx in range(3):
                        rr = src[:, b, r0 + 2:r0 + 2 + HB, kx:kx + W]
                        nc.tensor.matmul(
                            ps[:, k, 0:CHUNK], wr[:, kx, :], rr,
                            start=False, stop=(kx == 2))
                nc.vector.tensor_scalar(
                    out=dst_y[:, g0 * CHUNK:(g0 + gn) * CHUNK],
                    in0=ps[:, 0:gn, 0:CHUNK], scalar1=1.0, scalar2=None,
                    op0=Op.mult)

            # ---- stage 1 ----
            def stage1(b):
                y = tmp.tile([C, H * W], I32, tag="y", bufs=2)
                conv(y, w1, (xh, xl), b, two_pass=two_pass1)
                # t = floor(y*sw1) = RNE(y*sw1 - 0.5)
                t = tmp.tile([C, H * W], I32, tag="t")
                nc.vector.tensor_scalar(out=t, in0=y[:], scalar1=A1,
                                        scalar2=0.5, op0=Op.mult,
                                        op1=Op.subtract)
                # ra = RNE(x*sc1 + bb1) + MAGIC   (ACT; fp32 add at ulp=1)
                ra = tmp.tile([C, H * W], F32, tag="ra")
                nc.scalar.activation(out=ra, in_=xfs[b][:], func=IDENT,
                                     bias=BB1C, scale=SC1)
                # v' = t*bw1 + ra  (= v + MAGIC, exact ints)
                vp = tmp.tile([C, H * W], F32, tag="vp")
                nc.vector.scalar_tensor_tensor(out=vp, in0=t[:], scalar=B1,
                                               in1=ra[:], op0=Op.mult,
                                               op1=Op.add)
                # u = RNE((v' - MAGIC)*s1)
                u = tmp.tile([C, H * W], I32, tag="u")
                nc.vector.tensor_scalar(out=u, in0=vp[:], scalar1=MAGIC,
                                        scalar2=S1, op0=Op.subtract,
                                        op1=Op.mult)
                # x1 = clip(u,-7,7) -> fp16 into padded buffer
                nc.vector.tensor_scalar(
                    out=x1p[:, b, 1:1 + H, 1:1 + W], in0=u[:],
                    scalar1=7.0, scalar2=-7.0, op0=Op.min, op1=Op.max)

            # ---- stage 2 (per-group elementwise to shorten the tail) ----
            def stage2(b):
                y2 = tmp.tile([C, H * W], I32, tag="y", bufs=2)
                t2 = tmp.tile([C, H * W], I32, tag="t")
                r2s = tmp.tile([C, H * W], F32, tag="ra")
                u2 = tmp.tile([C, H * W], I32, tag="vp")
                for g0, gn in GROUPS:
                    c0, c1 = g0 * CHUNK, (g0 + gn) * CHUNK
                    rr0, rr1 = g0 * HB, (g0 + gn) * HB
                    conv2_group(y2, w2d, w2r, x1p, b, g0, gn)
                    nc.vector.tensor_scalar(
                        out=t2[:, c0:c1], in0=y2[:, c0:c1], scalar1=A2,
                        scalar2=0.5, op0=Op.mult, op1=Op.subtract)
                    # r2s = x1*(sc2*s2) + bb2*s2   (ACT)
                    nc.scalar.activation(
                        out=r2s[:, c0:c1],
                        in_=x1p[:, b, 1 + rr0:1 + rr1, 1:1 + W],
                        func=IDENT, bias=RBI, scale=RSC)
                    # u2 = RNE(t2*(bw2*s2) + r2s)  (fused, host-verified)
                    nc.vector.scalar_tensor_tensor(
                        out=u2[:, c0:c1], in0=t2[:, c0:c1], scalar=B2S,
                        in1=r2s[:, c0:c1], op0=Op.mult, op1=Op.add)
                    nc.vector.tensor_scalar(
                        out=out_sb[:, b, rr0:rr1, :], in0=u2[:, c0:c1],
                        scalar1=7.0, scalar2=-7.0, op0=Op.min, op1=Op.max)
                nc.sync.dma_start(out=d_o[:, b], in_=out_sb[:, b])

            # interleave: st2(b-1) after st1(b) so PE has conv2 work while
            # the last image's stage-1 elementwise chain drains
            for b in range(BS):
                stage1(b)
                if b > 0:
                    stage2(b - 1)
            stage2(BS - 1)

    nc.compile()
    _prog_cache[key] = nc
    return nc


# ---------------------------------------------------------------------------
# Entry point
# ---------------------------------------------------------------------------

last_results = None


def kernel(x, w1, w2, gamma1, beta1, mean1, var1,
           gamma2, beta2, mean2, var2):
    global last_results
    w1t, w2d, w2r, cv = _host_prep(x, w1, w2, gamma1, beta1, mean1, var1,
                              gamma2, beta2, mean2, var2)
    nc = _build_program()

    in_maps = []
    for i in range(NCORES):
        shard = np.ascontiguousarray(
            x[i * BS:(i + 1) * BS].astype(f32).transpose(1, 0, 2, 3))
        in_maps.append({"xt": shard, "w1s": w1t, "w2d": w2d,
                        "w2r": w2r, "cv": cv})

    trace = bool(int(os.environ.get("KERNEL_TRACE", "0")))
    kwargs = {}
    if trace:
        import concourse.bass_utils as _bu
        _bu.upload_artifacts = lambda tmpdir: ""
        kwargs["tmpdir"] = os.environ.get("KERNEL_TRACE_DIR", "/tmp/ktrace")
        os.makedirs(kwargs["tmpdir"], exist_ok=True)
    res = run_bass_kernel_spmd(nc, in_maps, core_ids=list(range(NCORES)),
                               trace=trace, **kwargs)
    last_results = res

    out = np.empty((B, C, H, W), np.float32)
    for i in range(NCORES):
        out[i * BS:(i + 1) * BS] = \
            res.results[i]["ot"].astype(np.float32).transpose(1, 0, 2, 3)
    return out
